# revision 2
# baseline (speedup 1.0000x reference)
"""Trainium2 Bass kernel for DeepInterestNetwork (DIN).

8 cores, data-parallel over batch; each core: 512 rows = 16 tiles of G=32.
Embedding rows are gathered host-side into two SBUF layouts (g-major
transposed stack for the attention matmul; slot layout for pooling); all
MLP/softmax/pooling compute runs on-device.

Key structure (vs. the naive per-row formulation):
  - attention layer1 folds to one K=128 matmul per row via
      inter @ W1 = q@(Wq+Wd) + k@(Wk-Wd) + (q*k)@Wm
    (the q term becomes a per-row-pair bias applied during the relu drain;
    the q*k rows are built on-device by one cross-partition-offset DVE
    multiply that reads the k half and writes the q*k half, so only half
    the stack is loaded from HBM).
  - kqt is g-major so every matmul RHS slice is contiguous in SBUF.
  - scores: 16 accumulating M=32 matmuls with per-pair column-selector
    stationaries (w2sel) build the full [32, T] score block in one PSUM
    bank -> a single Exp per tile whose accum_out yields the softmax
    denominators for free (scores are tiny, no max-subtraction needed).
  - softmax scatter to the pooling slot layout via 4 stream_shuffles
    (no DMA round-trip).
  - pooling: slot layout [q*32+g, e, r] keeps both the broadcast multiply
    and the X-axis reduction contiguous (DVE 2x-eligible); a pairing
    matmul folds partition quarters and transposes for the head MLP.
  - head MLP is batched across all 16 tiles at the end (3 matmuls total).
"""

import numpy as np
import sys

for p in ("/opt/trn_rl_repo", "/opt/trn_rl_repo/concourse"):
    if p not in sys.path:
        sys.path.insert(0, p)

VOCAB, E = 100000, 64
B, T = 4096, 200
NCORES = 8
BC = B // NCORES          # 512 rows per core
G = 32                    # batch rows per tile
NTILES = BC // G          # 16
NQ = 4                    # slot quarters: p = q*32 + g
RANKS = T // NQ           # 50
TOK = G * T               # 6400 columns per tile
GA = 22                   # qmul g-split: DVE does [0:GA), gpsimd [GA:32)
RELU_DVE = (2, 5, 8, 11, 14)  # row pairs whose relu runs on DVE

_nc_cache = {}


def build_nc(ntiles=NTILES):
    import concourse.bacc as bacc
    import concourse.mybir as mybir
    import concourse.tile as tile

    f32 = mybir.dt.float32
    f16 = mybir.dt.float16
    AF = mybir.ActivationFunctionType
    ALU = mybir.AluOpType
    AX = mybir.AxisListType

    nc = bacc.Bacc("TRN2", target_bir_lowering=False, debug=False)

    kqt_d = nc.dram_tensor("kqt", [ntiles, E, TOK], f16, kind="ExternalInput")
    stg_d = nc.dram_tensor("stg", [ntiles, 128, E, RANKS], f16, kind="ExternalInput")
    qtd_d = nc.dram_tensor("qtd", [ntiles, E, G, 2], f16, kind="ExternalInput")
    qto_d = nc.dram_tensor("qto", [ntiles, 65, G], f16, kind="ExternalInput")
    w1dbl_d = nc.dram_tensor("w1dbl", [128, 128], f16, kind="ExternalInput")
    wqdb_d = nc.dram_tensor("wqdb", [65, 128], f16, kind="ExternalInput")
    w2sel_d = nc.dram_tensor("w2sel", [128, G // 2, G], f16, kind="ExternalInput")
    dw1_d = nc.dram_tensor("dw1", [E, 128], f16, kind="ExternalInput")
    db1_d = nc.dram_tensor("db1", [128, 1], f32, kind="ExternalInput")
    dw2_d = nc.dram_tensor("dw2", [128, E], f16, kind="ExternalInput")
    db2_d = nc.dram_tensor("db2", [E, 1], f32, kind="ExternalInput")
    ow_d = nc.dram_tensor("ow", [E, 1], f16, kind="ExternalInput")
    ob_d = nc.dram_tensor("ob", [1, 1], f32, kind="ExternalInput")
    p2_d = nc.dram_tensor("p2", [128, G], f16, kind="ExternalInput")
    out_d = nc.dram_tensor("out", [1, ntiles * G], f32, kind="ExternalOutput")

    idmask = list(range(32))

    with tile.TileContext(nc) as tc:
        with tc.tile_pool(name="consts", bufs=1) as consts, \
             tc.tile_pool(name="kqtp", bufs=3) as kqt_pool, \
             tc.tile_pool(name="stgp", bufs=3) as stg_pool, \
             tc.tile_pool(name="work", bufs=2) as work_pool, \
             tc.tile_pool(name="hp", bufs=2) as h_pool, \
             tc.tile_pool(name="ph", bufs=3, space="PSUM") as ph_pool, \
             tc.tile_pool(name="sc", bufs=2, space="PSUM") as sc_pool, \
             tc.tile_pool(name="pm", bufs=2, space="PSUM") as pm_pool:

            # ---- constants ----
            w1dbl = consts.tile([128, 128], f16)
            nc.sync.dma_start(w1dbl[:], w1dbl_d.ap())
            wqdb = consts.tile([65, 128], f16)
            nc.sync.dma_start(wqdb[:], wqdb_d.ap())
            w2sel = consts.tile([128, G // 2, G], f16)
            nc.sync.dma_start(w2sel[:], w2sel_d.ap())
            dw1 = consts.tile([E, 128], f16)
            nc.sync.dma_start(dw1[:], dw1_d.ap())
            db1 = consts.tile([128, 1], f32)
            nc.sync.dma_start(db1[:], db1_d.ap())
            dw2 = consts.tile([128, E], f16)
            nc.sync.dma_start(dw2[:], dw2_d.ap())
            db2 = consts.tile([E, 1], f32)
            nc.sync.dma_start(db2[:], db2_d.ap())
            ow = consts.tile([E, 1], f16)
            nc.sync.dma_start(ow[:], ow_d.ap())
            ob = consts.tile([1, 1], f32)
            nc.sync.dma_start(ob[:], ob_d.ap())
            p2m = consts.tile([128, G], f16)
            nc.sync.dma_start(p2m[:], p2_d.ap())
            it512 = consts.tile([E, ntiles * G], f16, tag="it512")

            for ti in range(ntiles):
                # ---- load per-tile data (k half only; q*k built on-chip) ----
                kqt = kqt_pool.tile([128, TOK], f16, tag="kqt")
                nc.sync.dma_start(kqt[0:E, :], kqt_d.ap()[ti])
                stg2 = stg_pool.tile([128, E, RANKS], f16, tag="stg")
                nc.sync.dma_start(stg2[:], stg_d.ap()[ti])
                qtd = work_pool.tile([E, G, 2], f16, tag="qtd")
                nc.sync.dma_start(qtd[:], qtd_d.ap()[ti])
                qto = work_pool.tile([65, G], f16, tag="qto")
                nc.sync.dma_start(qto[:], qto_d.ap()[ti])

                # build q*k rows in the lower half: kqt[64+e, (g,t)] =
                # kqt[e, (g,t)] * q[e, g]; pair-duplicated qtd keeps the
                # broadcast AP innermost-contiguous (DVE 2x mode).
                kqv = kqt[:].rearrange("p (g t2 d) -> p g t2 d", g=G, d=2)
                nc.vector.tensor_mul(
                    kqv[E:128, 0:GA],
                    kqv[0:E, 0:GA],
                    qtd[:, 0:GA, None, :].broadcast_to([E, GA, T // 2, 2]),
                )
                nc.gpsimd.tensor_mul(
                    kqv[E:128, GA:G],
                    kqv[0:E, GA:G],
                    qtd[:, GA:G, None, :].broadcast_to([E, G - GA, T // 2, 2]),
                )

                # ---- per-row-pair bias c = q @ (Wq+Wd) + b1 -> ctd [128, 16]
                qtov = qto[:].rearrange("p (g two) -> p two g", two=2)
                pctd = pm_pool.tile([128, G // 2], f32, tag="pm")
                nc.tensor.matmul(
                    pctd[0:64, :], wqdb[:, 0:64], qtov[:, 0, :],
                    tile_position=(0, 0),
                )
                nc.tensor.matmul(
                    pctd[64:128, :], wqdb[:, 64:128], qtov[:, 1, :],
                    tile_position=(0, 64),
                )
                ctd = work_pool.tile([128, G // 2], f32, tag="ctd")
                nc.scalar.copy(ctd[:], pctd[:])

                # ---- attention MLP + scores ----
                kqt3 = kqt[:].rearrange("p (g t) -> p g t", g=G)
                h16 = h_pool.tile([128, G // 2, T], f16, tag="h16")
                scores = sc_pool.tile([G, T], f32, tag="sc")
                for j in range(G // 2):
                    ph = ph_pool.tile([128, T], f32, tag="ph")
                    nc.tensor.matmul(
                        ph[0:64, :], w1dbl[:, 0:64], kqt3[:, 2 * j, :],
                        tile_position=(0, 0),
                    )
                    nc.tensor.matmul(
                        ph[64:128, :], w1dbl[:, 64:128], kqt3[:, 2 * j + 1, :],
                        tile_position=(0, 64),
                    )
                    if j in RELU_DVE:
                        nc.vector.tensor_scalar(
                            h16[:, j, :], ph[:], ctd[:, j:j + 1], 0.0,
                            ALU.add, ALU.max,
                        )
                    else:
                        nc.scalar.activation(
                            h16[:, j, :], ph[:], AF.Relu, bias=ctd[:, j:j + 1],
                        )
                    nc.tensor.matmul(
                        scores[:], w2sel[:, j, :], h16[:, j, :],
                        start=(j == 0), stop=(j == G // 2 - 1),
                        skip_group_check=True,
                    )

                # ---- softmax (exp + accum denominators + normalize) ----
                es32 = work_pool.tile([G, T], f16, tag="es32")
                dsum = work_pool.tile([G, 1], f32, tag="dsum")
                nc.scalar.activation(es32[:], scores[:], AF.Exp, accum_out=dsum[:])
                rsum = work_pool.tile([G, 1], f32, tag="rsum")
                nc.vector.reciprocal(rsum[:], dsum[:])
                nc.vector.tensor_scalar_mul(es32[:], es32[:], rsum[:])

                # scatter to slot layout esp[q*32+g, r] = es32[g, 4r+q]
                esp = work_pool.tile([128, RANKS], f16, tag="esp")
                esv = es32[:].rearrange("p (r q) -> p q r", q=NQ)
                for q in range(NQ):
                    nc.vector.stream_shuffle(
                        esp[q * G:(q + 1) * G, :], esv[:, q, :], idmask
                    )

                # ---- pooling: interest = sum_t es * k (slot layout) ----
                tmp = work_pool.tile([128, E, RANKS], f16, tag="tmp")
                nc.vector.tensor_mul(
                    tmp[:], stg2[:],
                    esp[:, None, :].broadcast_to([128, E, RANKS]),
                )
                acc = work_pool.tile([128, E], f16, tag="acc")
                with nc.allow_low_precision("f16 pooling accum, tol 2e-2"):
                    nc.vector.tensor_reduce(acc[:, :, None], tmp[:], AX.X, ALU.add)

                # fold quarters + transpose via pairing matmul
                pit = pm_pool.tile([E, G], f32, tag="pm")
                nc.tensor.matmul(pit[:], acc[:], p2m[:])
                nc.scalar.copy(it512[:, ti * G:(ti + 1) * G], pit[:])

            # ---- head MLP, batched over all tiles ----
            pd1 = ph_pool.tile([128, ntiles * G], f32, tag="ph")
            nc.tensor.matmul(pd1[:], dw1[:], it512[:])
            d1h = h_pool.tile([128, ntiles * G], f16, tag="h16")
            nc.scalar.activation(d1h[:], pd1[:], AF.Relu, bias=db1[:])
            pd2 = ph_pool.tile([E, ntiles * G], f32, tag="ph")
            nc.tensor.matmul(pd2[:], dw2[:], d1h[:])
            d2h = h_pool.tile([E, ntiles * G], f16, tag="h16")
            nc.scalar.activation(d2h[:], pd2[:], AF.Relu, bias=db2[:])
            po = pm_pool.tile([1, ntiles * G], f32, tag="pm")
            nc.tensor.matmul(po[:], ow[:], d2h[:])
            outb = work_pool.tile([1, ntiles * G], f32, tag="outb")
            nc.scalar.activation(outb[:], po[:], AF.Sigmoid, bias=ob[:])
            nc.sync.dma_start(out_d.ap(), outb[:])

    nc.compile()
    return nc


def marshal_inputs(query, keys, emb, att_w1, att_b1, att_w2, att_b2,
                   deep_w1, deep_b1, deep_w2, deep_b2, out_w, out_b,
                   ntiles=NTILES):
    query = np.asarray(query).astype(np.int64)
    keys = np.asarray(keys).astype(np.int64)
    emb16 = np.asarray(emb, dtype=np.float32).astype(np.float16)
    a1 = np.asarray(att_w1, dtype=np.float32)
    Wq, Wk, Wd, Wm = a1[0:64], a1[64:128], a1[128:192], a1[192:256]
    Wpp = np.vstack([Wk - Wd, Wm])
    w1dbl = np.hstack([Wpp, Wpp]).astype(np.float16)
    aug = np.vstack([Wq + Wd, np.asarray(att_b1, np.float32)[None, :]])
    wqdb = np.hstack([aug, aug]).astype(np.float16)
    w2 = np.asarray(att_w2, np.float32)[:, 0]
    # w2sel[:, j, 2j] = [w2; 0], w2sel[:, j, 2j+1] = [0; w2]
    w2sel = np.zeros((128, G // 2, G), np.float16)
    for j in range(G // 2):
        w2sel[0:64, j, 2 * j] = w2
        w2sel[64:128, j, 2 * j + 1] = w2
    dw1 = np.asarray(deep_w1, np.float32).astype(np.float16)
    db1 = np.asarray(deep_b1, np.float32).reshape(128, 1)
    dw2 = np.asarray(deep_w2, np.float32).astype(np.float16)
    db2 = np.asarray(deep_b2, np.float32).reshape(64, 1)
    ow = np.asarray(out_w, np.float32).astype(np.float16)
    ob = np.asarray(out_b, np.float32).reshape(1, 1)
    p2 = (np.arange(128)[:, None] % G == np.arange(G)[None, :]).astype(np.float16)

    in_maps = []
    for c in range(NCORES):
        rows = slice(c * BC, c * BC + ntiles * G)
        kt = emb16[keys[rows]].reshape(ntiles, G, T, E)       # [nt,G,T,E]
        qe = emb16[query[rows]].reshape(ntiles, G, E)         # [nt,G,E]
        # g-major transposed stack: kqt[ti, e, g*T+t] = kt[ti, g, t, e]
        kqt = np.ascontiguousarray(
            kt.transpose(0, 3, 1, 2).reshape(ntiles, E, TOK)
        )
        # slot layout: stg[ti, q*32+g, e, r] = kt[ti, g, 4r+q, e]
        ktr = kt.reshape(ntiles, G, RANKS, NQ, E)
        stg = np.ascontiguousarray(
            ktr.transpose(0, 3, 1, 4, 2).reshape(ntiles, 128, E, RANKS)
        )
        qT = qe.transpose(0, 2, 1)                            # [nt,E,G]
        qtd = np.ascontiguousarray(
            np.repeat(qT[:, :, :, None], 2, axis=3)           # [nt,E,G,2]
        )
        qto = np.ascontiguousarray(np.concatenate(
            [qT, np.ones((ntiles, 1, G), np.float16)], axis=1))
        in_maps.append({
            "kqt": kqt, "stg": stg, "qtd": qtd, "qto": qto,
            "w1dbl": w1dbl, "wqdb": wqdb, "w2sel": w2sel,
            "dw1": dw1, "db1": db1, "dw2": dw2, "db2": db2,
            "ow": ow, "ob": ob, "p2": p2,
        })
    return in_maps


def kernel(**inputs) -> np.ndarray:
    from concourse.bass_utils import run_bass_kernel_spmd

    if "full" not in _nc_cache:
        _nc_cache["full"] = build_nc(NTILES)
    nc = _nc_cache["full"]
    in_maps = marshal_inputs(**inputs)
    res = run_bass_kernel_spmd(nc, in_maps, core_ids=list(range(NCORES)))
    outs = [res.results[c]["out"].reshape(-1) for c in range(NCORES)]
    return np.concatenate(outs).reshape(B, 1).astype(np.float32)


if __name__ == "__main__":
    sys.path.insert(0, "/root/problem")
    import reference
    inputs = {k: np.asarray(v) for k, v in reference.setup_inputs().items()}
    expected = np.asarray(reference.reference(**inputs))
    actual = kernel(**inputs)
    err = np.abs(actual - expected).max() / (np.abs(expected).max() + 1e-12)
    print("Relative error:", err)


# revision 3
# speedup vs baseline: 1.3862x; 1.3862x over previous
"""Trainium2 Bass kernel for DeepInterestNetwork (DIN).

8 cores, data-parallel over batch; each core: 512 rows = 16 tiles of G=32.
Embedding rows are gathered host-side into two SBUF layouts; all
MLP/softmax/pooling compute runs on-device.

Key structure:
  - attention layer1 folds to one K=128 matmul per row via
      inter @ W1 = q@(Wq+Wd) + k@(Wk-Wd) + (q*k)@Wm
    (the q term becomes a per-row-pair bias applied during the relu drain;
    the q*k half of the stack is premultiplied host-side).
  - kqt columns are g-major with even rows first, so each w1 matmul streams
    N=400 (two same-parity rows) with one stationary; a [128, 400] PSUM bank
    then holds a natural row pair per 200-column half for the relu drain.
  - scores: 16 accumulating M=32 matmuls with per-pair column-selector
    stationaries (w2sel) build the full [32, T] score block in one PSUM
    bank -> a single Exp per tile whose accum_out yields the softmax
    denominators for free (scores are tiny, no max-subtraction needed).
  - softmax scatter to the pooling slot layout via 4 stream_shuffles.
  - pooling: slot layout [q*32+g, e, r] keeps the broadcast multiply and the
    X-axis reduction contiguous; the multiply is split DVE/GPSIMD; a pairing
    matmul folds partition quarters and transposes for the head MLP.
  - per-row-pair biases (ctd) are computed for all 16 tiles in one batched
    pre-pass; the head MLP is batched across all 16 tiles at the end.
"""

import numpy as np
import sys

for p in ("/opt/trn_rl_repo", "/opt/trn_rl_repo/concourse"):
    if p not in sys.path:
        sys.path.insert(0, p)

VOCAB, E = 100000, 64
B, T = 4096, 200
NCORES = 8
BC = B // NCORES          # 512 rows per core
G = 32                    # batch rows per tile
NTILES = BC // G          # 16
NQ = 4                    # slot quarters: p = q*32 + g
RANKS = T // NQ           # 50
TOK = G * T               # 6400 columns per tile
EA = 40                   # pool-mul split: DVE e[0:EA), gpsimd e[EA:64)
RELU_DVE = (3, 8, 13)     # row pairs whose relu runs on DVE

# even-rows-first marshaling order
PI = [2 * i for i in range(16)] + [2 * i + 1 for i in range(16)]

_nc_cache = {}


def build_nc(ntiles=NTILES):
    import concourse.bacc as bacc
    import concourse.mybir as mybir
    import concourse.tile as tile

    f32 = mybir.dt.float32
    f16 = mybir.dt.float16
    AF = mybir.ActivationFunctionType
    ALU = mybir.AluOpType
    AX = mybir.AxisListType

    nc = bacc.Bacc("TRN2", target_bir_lowering=False, debug=False)

    kqt_d = nc.dram_tensor("kqt", [ntiles, 128, TOK], f16, kind="ExternalInput")
    stg_d = nc.dram_tensor("stg", [ntiles, 128, E, RANKS], f16, kind="ExternalInput")
    qta_d = nc.dram_tensor("qta", [65, 2, ntiles * G // 2], f16, kind="ExternalInput")
    w1dbl_d = nc.dram_tensor("w1dbl", [128, 128], f16, kind="ExternalInput")
    wqdb_d = nc.dram_tensor("wqdb", [65, 128], f16, kind="ExternalInput")
    w2sel_d = nc.dram_tensor("w2sel", [128, G // 2, G], f16, kind="ExternalInput")
    dw1_d = nc.dram_tensor("dw1", [E, 128], f16, kind="ExternalInput")
    db1_d = nc.dram_tensor("db1", [128, 1], f32, kind="ExternalInput")
    dw2_d = nc.dram_tensor("dw2", [128, E], f16, kind="ExternalInput")
    db2_d = nc.dram_tensor("db2", [E, 1], f32, kind="ExternalInput")
    ow_d = nc.dram_tensor("ow", [E, 1], f16, kind="ExternalInput")
    ob_d = nc.dram_tensor("ob", [1, 1], f32, kind="ExternalInput")
    p2_d = nc.dram_tensor("p2", [128, G], f16, kind="ExternalInput")
    out_d = nc.dram_tensor("out", [1, ntiles * G], f32, kind="ExternalOutput")

    idmask = list(range(32))
    NC2 = ntiles * G // 2     # 256 row pairs per core

    with tile.TileContext(nc) as tc:
        with tc.tile_pool(name="consts", bufs=1) as consts, \
             tc.tile_pool(name="kqtp", bufs=3) as kqt_pool, \
             tc.tile_pool(name="stgp", bufs=3) as stg_pool, \
             tc.tile_pool(name="work", bufs=2) as work_pool, \
             tc.tile_pool(name="hp", bufs=2) as h_pool, \
             tc.tile_pool(name="ph", bufs=3, space="PSUM") as ph_pool, \
             tc.tile_pool(name="sc", bufs=2, space="PSUM") as sc_pool, \
             tc.tile_pool(name="pm", bufs=2, space="PSUM") as pm_pool:

            # ---- constants ----
            w1dbl = consts.tile([128, 128], f16)
            nc.sync.dma_start(w1dbl[:], w1dbl_d.ap())
            wqdb = consts.tile([65, 128], f16)
            nc.sync.dma_start(wqdb[:], wqdb_d.ap())
            w2sel = consts.tile([128, G // 2, G], f16)
            nc.sync.dma_start(w2sel[:], w2sel_d.ap())
            dw1 = consts.tile([E, 128], f16)
            nc.sync.dma_start(dw1[:], dw1_d.ap())
            db1 = consts.tile([128, 1], f32)
            nc.sync.dma_start(db1[:], db1_d.ap())
            dw2 = consts.tile([128, E], f16)
            nc.sync.dma_start(dw2[:], dw2_d.ap())
            db2 = consts.tile([E, 1], f32)
            nc.sync.dma_start(db2[:], db2_d.ap())
            ow = consts.tile([E, 1], f16)
            nc.sync.dma_start(ow[:], ow_d.ap())
            ob = consts.tile([1, 1], f32)
            nc.sync.dma_start(ob[:], ob_d.ap())
            p2m = consts.tile([128, G], f16)
            nc.sync.dma_start(p2m[:], p2_d.ap())
            qta = consts.tile([65, 2, NC2], f16)
            nc.sync.dma_start(qta[:], qta_d.ap())
            it512 = consts.tile([E, ntiles * G], f16, tag="it512")

            # ---- batched bias pre-pass: ctd_all[:, ti*16+j] = c(pair j) ----
            pca = pm_pool.tile([128, NC2], f32, tag="pm")
            nc.tensor.matmul(
                pca[0:64, :], wqdb[:, 0:64], qta[:, 0, :], tile_position=(0, 0)
            )
            nc.tensor.matmul(
                pca[64:128, :], wqdb[:, 64:128], qta[:, 1, :],
                tile_position=(0, 64),
            )
            ctd_all = consts.tile([128, NC2], f32, tag="ctd")
            nc.scalar.copy(ctd_all[:], pca[:])

            for ti in range(ntiles):
                kqt = kqt_pool.tile([128, TOK], f16, tag="kqt")
                nc.sync.dma_start(kqt[:], kqt_d.ap()[ti])
                stg2 = stg_pool.tile([128, E, RANKS], f16, tag="stg")
                nc.sync.dma_start(stg2[:], stg_d.ap()[ti])

                # ---- attention MLP + scores ----
                # kqt columns: rows in PI order (16 even rows then 16 odd),
                # g-major. w1 matmul k streams two same-parity rows (N=400);
                # psum bank half c holds the natural pair (4k+2c, 4k+2c+1).
                h16 = h_pool.tile([128, G // 2, T], f16, tag="h16")
                scores = sc_pool.tile([G, T], f32, tag="sc")
                for k in range(8):
                    ph = ph_pool.tile([128, 2, T], f32, tag="ph")
                    nc.tensor.matmul(
                        ph[0:64, :, :], w1dbl[:, 0:64],
                        kqt[:, 2 * T * k:2 * T * (k + 1)],
                        tile_position=(0, 0),
                    )
                    nc.tensor.matmul(
                        ph[64:128, :, :], w1dbl[:, 64:128],
                        kqt[:, TOK // 2 + 2 * T * k:TOK // 2 + 2 * T * (k + 1)],
                        tile_position=(0, 64),
                    )
                    for c in range(2):
                        j = 2 * k + c
                        bias = ctd_all[:, ti * (G // 2) + j:ti * (G // 2) + j + 1]
                        if j in RELU_DVE:
                            nc.vector.tensor_scalar(
                                h16[:, j, :], ph[:, c, :], bias, 0.0,
                                ALU.add, ALU.max,
                            )
                        else:
                            nc.scalar.activation(
                                h16[:, j, :], ph[:, c, :], AF.Relu, bias=bias,
                            )
                        nc.tensor.matmul(
                            scores[:], w2sel[:, j, :], h16[:, j, :],
                            start=(j == 0), stop=(j == G // 2 - 1),
                            skip_group_check=True,
                        )

                # ---- softmax (exp + accum denominators + normalize) ----
                es32 = work_pool.tile([G, T], f16, tag="es32")
                dsum = work_pool.tile([G, 1], f32, tag="dsum")
                nc.scalar.activation(es32[:], scores[:], AF.Exp, accum_out=dsum[:])
                rsum = work_pool.tile([G, 1], f32, tag="rsum")
                nc.vector.reciprocal(rsum[:], dsum[:])
                nc.vector.tensor_scalar_mul(es32[:], es32[:], rsum[:])

                # scatter to slot layout esp[q*32+g, r] = es32[g, 4r+q]
                esp = work_pool.tile([128, RANKS], f16, tag="esp")
                esv = es32[:].rearrange("p (r q) -> p q r", q=NQ)
                for q in range(NQ):
                    nc.vector.stream_shuffle(
                        esp[q * G:(q + 1) * G, :], esv[:, q, :], idmask
                    )

                # ---- pooling: interest = sum_t es * k (slot layout) ----
                tmp = work_pool.tile([128, E, RANKS], f16, tag="tmp")
                nc.vector.tensor_mul(
                    tmp[:, 0:EA, :], stg2[:, 0:EA, :],
                    esp[:, None, :].broadcast_to([128, EA, RANKS]),
                )
                nc.gpsimd.tensor_mul(
                    tmp[:, EA:E, :], stg2[:, EA:E, :],
                    esp[:, None, :].broadcast_to([128, E - EA, RANKS]),
                )
                acc = work_pool.tile([128, E], f16, tag="acc")
                with nc.allow_low_precision("f16 pooling accum, tol 2e-2"):
                    nc.vector.tensor_reduce(acc[:, :, None], tmp[:], AX.X, ALU.add)

                # fold quarters + transpose via pairing matmul
                pit = pm_pool.tile([E, G], f32, tag="pm")
                nc.tensor.matmul(pit[:], acc[:], p2m[:])
                nc.scalar.copy(it512[:, ti * G:(ti + 1) * G], pit[:])

            # ---- head MLP, batched over all tiles ----
            pd1 = ph_pool.tile([128, ntiles * G], f32, tag="ph")
            nc.tensor.matmul(pd1[:], dw1[:], it512[:])
            d1h = h_pool.tile([128, ntiles * G], f16, tag="h16")
            nc.scalar.activation(d1h[:], pd1[:], AF.Relu, bias=db1[:])
            pd2 = ph_pool.tile([E, ntiles * G], f32, tag="ph")
            nc.tensor.matmul(pd2[:], dw2[:], d1h[:])
            d2h = h_pool.tile([E, ntiles * G], f16, tag="h16")
            nc.scalar.activation(d2h[:], pd2[:], AF.Relu, bias=db2[:])
            po = pm_pool.tile([1, ntiles * G], f32, tag="pm")
            nc.tensor.matmul(po[:], ow[:], d2h[:])
            outb = work_pool.tile([1, ntiles * G], f32, tag="outb")
            nc.scalar.activation(outb[:], po[:], AF.Sigmoid, bias=ob[:])
            nc.sync.dma_start(out_d.ap(), outb[:])

    nc.compile()
    return nc


def marshal_inputs(query, keys, emb, att_w1, att_b1, att_w2, att_b2,
                   deep_w1, deep_b1, deep_w2, deep_b2, out_w, out_b,
                   ntiles=NTILES):
    query = np.asarray(query).astype(np.int64)
    keys = np.asarray(keys).astype(np.int64)
    emb16 = np.asarray(emb, dtype=np.float32).astype(np.float16)
    a1 = np.asarray(att_w1, dtype=np.float32)
    Wq, Wk, Wd, Wm = a1[0:64], a1[64:128], a1[128:192], a1[192:256]
    Wpp = np.vstack([Wk - Wd, Wm])
    w1dbl = np.hstack([Wpp, Wpp]).astype(np.float16)
    aug = np.vstack([Wq + Wd, np.asarray(att_b1, np.float32)[None, :]])
    wqdb = np.hstack([aug, aug]).astype(np.float16)
    w2 = np.asarray(att_w2, np.float32)[:, 0]
    w2sel = np.zeros((128, G // 2, G), np.float16)
    for j in range(G // 2):
        w2sel[0:64, j, 2 * j] = w2
        w2sel[64:128, j, 2 * j + 1] = w2
    dw1 = np.asarray(deep_w1, np.float32).astype(np.float16)
    db1 = np.asarray(deep_b1, np.float32).reshape(128, 1)
    dw2 = np.asarray(deep_w2, np.float32).astype(np.float16)
    db2 = np.asarray(deep_b2, np.float32).reshape(64, 1)
    ow = np.asarray(out_w, np.float32).astype(np.float16)
    ob = np.asarray(out_b, np.float32).reshape(1, 1)
    p2 = (np.arange(128)[:, None] % G == np.arange(G)[None, :]).astype(np.float16)

    in_maps = []
    for c in range(NCORES):
        rows = slice(c * BC, c * BC + ntiles * G)
        kt = emb16[keys[rows]].reshape(ntiles, G, T, E)       # [nt,G,T,E]
        qe = emb16[query[rows]].reshape(ntiles, G, E)         # [nt,G,E]
        # kqt[ti, e, gi*T+t] = kt[ti, PI[gi], t, e] (top), * q (bottom)
        ktp = kt[:, PI, :, :]                                 # [nt,32,T,E]
        qep = qe[:, PI, :]                                    # [nt,32,E]
        kh = ktp.transpose(0, 3, 1, 2)                        # [nt,E,32,T]
        kqh = kh * qep.transpose(0, 2, 1)[:, :, :, None]      # *q[e,gi]
        kqt = np.ascontiguousarray(np.concatenate(
            [kh.reshape(ntiles, E, TOK), kqh.reshape(ntiles, E, TOK)], axis=1))
        # slot layout: stg[ti, q*32+g, e, r] = kt[ti, g, 4r+q, e]
        ktr = kt.reshape(ntiles, G, RANKS, NQ, E)
        stg = np.ascontiguousarray(
            ktr.transpose(0, 3, 1, 4, 2).reshape(ntiles, 128, E, RANKS)
        )
        # batched bias rhs: qta[:, par, ti*16+j] = [q(2j+par); 1]
        qT = qe.transpose(0, 2, 1)                            # [nt,E,G]
        qaug = np.concatenate([qT, np.ones((ntiles, 1, G), np.float16)], 1)
        qta = np.ascontiguousarray(
            qaug.reshape(ntiles, 65, G // 2, 2).transpose(1, 3, 0, 2)
            .reshape(65, 2, ntiles * G // 2)
        )
        in_maps.append({
            "kqt": kqt, "stg": stg, "qta": qta,
            "w1dbl": w1dbl, "wqdb": wqdb, "w2sel": w2sel,
            "dw1": dw1, "db1": db1, "dw2": dw2, "db2": db2,
            "ow": ow, "ob": ob, "p2": p2,
        })
    return in_maps


def kernel(**inputs) -> np.ndarray:
    from concourse.bass_utils import run_bass_kernel_spmd

    if "full" not in _nc_cache:
        _nc_cache["full"] = build_nc(NTILES)
    nc = _nc_cache["full"]
    in_maps = marshal_inputs(**inputs)
    res = run_bass_kernel_spmd(nc, in_maps, core_ids=list(range(NCORES)))
    outs = [res.results[c]["out"].reshape(-1) for c in range(NCORES)]
    return np.concatenate(outs).reshape(B, 1).astype(np.float32)


if __name__ == "__main__":
    sys.path.insert(0, "/root/problem")
    import reference
    inputs = {k: np.asarray(v) for k, v in reference.setup_inputs().items()}
    expected = np.asarray(reference.reference(**inputs))
    actual = kernel(**inputs)
    err = np.abs(actual - expected).max() / (np.abs(expected).max() + 1e-12)
    print("Relative error:", err)


# revision 6
# speedup vs baseline: 1.6340x; 1.1788x over previous
"""Trainium2 Bass kernel for DeepInterestNetwork (DIN).

8 cores, data-parallel over batch; each core: 512 rows = 16 tiles of G=32.
Embedding rows are gathered host-side into two SBUF layouts; all
MLP/softmax/pooling compute runs on-device.

Key structure:
  - attention layer1 folds to one K=128 matmul per row via
      inter @ W1 = q@(Wq+Wd) + k@(Wk-Wd) + (q*k)@Wm
    (the q term becomes a per-row-pair bias applied during the relu drain;
    the q*k half of the stack is premultiplied host-side).
  - kqt columns are g-major with even rows first, so each w1 matmul streams
    N=400 (two same-parity rows) with one stationary; a [128, 400] PSUM bank
    then holds a natural row pair per 200-column half for the relu drain.
  - scores: 16 accumulating M=32 matmuls with per-pair column-selector
    stationaries (w2sel) build the full [32, T] score block in one PSUM
    bank -> a single Exp per tile whose accum_out yields the softmax
    denominators for free (scores are tiny, no max-subtraction needed).
  - softmax scatter to the pooling slot layout via 4 stream_shuffles.
  - pooling: slot layout [q*32+g, e, r] keeps the broadcast multiply and the
    X-axis reduction contiguous; the multiply is split DVE/GPSIMD; a pairing
    matmul folds partition quarters and transposes for the head MLP.
  - per-row-pair biases (ctd) are computed for all 16 tiles in one batched
    pre-pass; the head MLP is batched across all 16 tiles at the end.
"""

import numpy as np
import sys

for p in ("/opt/trn_rl_repo", "/opt/trn_rl_repo/concourse"):
    if p not in sys.path:
        sys.path.insert(0, p)

VOCAB, E = 100000, 64
B, T = 4096, 200
NCORES = 8
BC = B // NCORES          # 512 rows per core
G = 32                    # batch rows per tile
NTILES = BC // G          # 16
NQ = 4                    # slot quarters: p = q*32 + g
RANKS = T // NQ           # 50
TOK = G * T               # 6400 columns per tile
RA = 34                   # pool-mul split: DVE r[0:RA), gpsimd r[RA:50)
RELU_DVE = (3, 8, 13)     # row pairs whose relu runs on DVE

# even-rows-first marshaling order
PI = [2 * i for i in range(16)] + [2 * i + 1 for i in range(16)]

_nc_cache = {}


def build_nc(ntiles=NTILES):
    import concourse.bacc as bacc
    import concourse.mybir as mybir
    import concourse.tile as tile

    f32 = mybir.dt.float32
    f16 = mybir.dt.float16
    AF = mybir.ActivationFunctionType
    ALU = mybir.AluOpType
    AX = mybir.AxisListType

    nc = bacc.Bacc("TRN2", target_bir_lowering=False, debug=False)

    kqt_d = nc.dram_tensor("kqt", [ntiles, 128, TOK], f16, kind="ExternalInput")
    stg_d = nc.dram_tensor("stg", [ntiles, 128, E, RANKS], f16, kind="ExternalInput")
    qta_d = nc.dram_tensor("qta", [65, 2, ntiles * G // 2], f16, kind="ExternalInput")
    w1dbl_d = nc.dram_tensor("w1dbl", [128, 128], f16, kind="ExternalInput")
    wqdb_d = nc.dram_tensor("wqdb", [65, 128], f16, kind="ExternalInput")
    w2sel_d = nc.dram_tensor("w2sel", [128, G // 2, G], f16, kind="ExternalInput")
    dw1_d = nc.dram_tensor("dw1", [E, 128], f16, kind="ExternalInput")
    db1_d = nc.dram_tensor("db1", [128, 1], f32, kind="ExternalInput")
    dw2_d = nc.dram_tensor("dw2", [128, E], f16, kind="ExternalInput")
    db2_d = nc.dram_tensor("db2", [E, 1], f32, kind="ExternalInput")
    ow_d = nc.dram_tensor("ow", [E, 1], f16, kind="ExternalInput")
    ob_d = nc.dram_tensor("ob", [1, 1], f32, kind="ExternalInput")
    p2_d = nc.dram_tensor("p2", [128, G], f16, kind="ExternalInput")
    out_d = nc.dram_tensor("out", [1, ntiles * G], f32, kind="ExternalOutput")

    idmask = list(range(32))
    NC2 = ntiles * G // 2     # 256 row pairs per core

    with tile.TileContext(nc) as tc:
        with tc.tile_pool(name="consts", bufs=1) as consts, \
             tc.tile_pool(name="kqtp", bufs=3) as kqt_pool, \
             tc.tile_pool(name="stgp", bufs=3) as stg_pool, \
             tc.tile_pool(name="work", bufs=2) as work_pool, \
             tc.tile_pool(name="hp", bufs=2) as h_pool, \
             tc.tile_pool(name="ph", bufs=4, space="PSUM") as ph_pool, \
             tc.tile_pool(name="sc", bufs=2, space="PSUM") as sc_pool, \
             tc.tile_pool(name="pm", bufs=2, space="PSUM") as pm_pool:

            # ---- constants ----
            w1dbl = consts.tile([128, 128], f16)
            nc.sync.dma_start(w1dbl[:], w1dbl_d.ap())
            wqdb = consts.tile([65, 128], f16)
            nc.sync.dma_start(wqdb[:], wqdb_d.ap())
            w2sel = consts.tile([128, G // 2, G], f16)
            nc.sync.dma_start(w2sel[:], w2sel_d.ap())
            dw1 = consts.tile([E, 128], f16)
            nc.sync.dma_start(dw1[:], dw1_d.ap())
            db1 = consts.tile([128, 1], f32)
            nc.sync.dma_start(db1[:], db1_d.ap())
            dw2 = consts.tile([128, E], f16)
            nc.sync.dma_start(dw2[:], dw2_d.ap())
            db2 = consts.tile([E, 1], f32)
            nc.sync.dma_start(db2[:], db2_d.ap())
            ow = consts.tile([E, 1], f16)
            nc.sync.dma_start(ow[:], ow_d.ap())
            ob = consts.tile([1, 1], f32)
            nc.sync.dma_start(ob[:], ob_d.ap())
            p2m = consts.tile([128, G], f16)
            nc.sync.dma_start(p2m[:], p2_d.ap())
            qta = consts.tile([65, 2, NC2], f16)
            nc.sync.dma_start(qta[:], qta_d.ap())
            it512 = consts.tile([E, ntiles * G], f16, tag="it512")

            # ---- batched bias pre-pass: ctd_all[:, ti*16+j] = c(pair j) ----
            pca = pm_pool.tile([128, NC2], f32, tag="pm")
            nc.tensor.matmul(
                pca[0:64, :], wqdb[:, 0:64], qta[:, 0, :], tile_position=(0, 0)
            )
            nc.tensor.matmul(
                pca[64:128, :], wqdb[:, 64:128], qta[:, 1, :],
                tile_position=(0, 64),
            )
            ctd_all = consts.tile([128, NC2], f32, tag="ctd")
            nc.scalar.copy(ctd_all[:], pca[:])

            for ti in range(ntiles):
                kqt = kqt_pool.tile([128, TOK], f16, tag="kqt")
                nc.sync.dma_start(kqt[:], kqt_d.ap()[ti])
                stg2 = stg_pool.tile([128, E, RANKS], f16, tag="stg")
                nc.sync.dma_start(stg2[:], stg_d.ap()[ti])

                # ---- attention MLP + scores ----
                # kqt columns: rows in PI order (16 even rows then 16 odd),
                # g-major. w1 matmul k streams two same-parity rows (N=400);
                # psum bank half c holds the natural pair (4k+2c, 4k+2c+1).
                h16 = h_pool.tile([128, G // 2, T], f16, tag="h16")
                scores = sc_pool.tile([G, T], f32, tag="sc")
                for k in range(8):
                    ph = ph_pool.tile([128, 2, T], f32, tag="ph")
                    nc.tensor.matmul(
                        ph[0:64, :, :], w1dbl[:, 0:64],
                        kqt[:, 2 * T * k:2 * T * (k + 1)],
                        tile_position=(0, 0),
                    )
                    nc.tensor.matmul(
                        ph[64:128, :, :], w1dbl[:, 64:128],
                        kqt[:, TOK // 2 + 2 * T * k:TOK // 2 + 2 * T * (k + 1)],
                        tile_position=(0, 64),
                    )
                    for c in range(2):
                        j = 2 * k + c
                        bias = ctd_all[:, ti * (G // 2) + j:ti * (G // 2) + j + 1]
                        if j in RELU_DVE:
                            nc.vector.tensor_scalar(
                                h16[:, j, :], ph[:, c, :], bias, 0.0,
                                ALU.add, ALU.max,
                            )
                        else:
                            nc.scalar.activation(
                                h16[:, j, :], ph[:, c, :], AF.Relu, bias=bias,
                            )
                        nc.tensor.matmul(
                            scores[:], w2sel[:, j, :], h16[:, j, :],
                            start=(j == 0), stop=(j == G // 2 - 1),
                            skip_group_check=True,
                        )

                # ---- softmax (exp + accum denominators + normalize) ----
                es32 = work_pool.tile([G, T], f16, tag="es32")
                dsum = work_pool.tile([G, 1], f32, tag="dsum")
                nc.scalar.activation(es32[:], scores[:], AF.Exp, accum_out=dsum[:])
                rsum = work_pool.tile([G, 1], f32, tag="rsum")
                nc.vector.reciprocal(rsum[:], dsum[:])
                nc.vector.tensor_scalar_mul(es32[:], es32[:], rsum[:])

                # scatter to slot layout esp[q*32+g, r] = es32[g, 4r+q]
                esp = work_pool.tile([128, RANKS], f16, tag="esp")
                esv = es32[:].rearrange("p (r q) -> p q r", q=NQ)
                for q in range(NQ):
                    nc.vector.stream_shuffle(
                        esp[q * G:(q + 1) * G, :], esv[:, q, :], idmask
                    )

                # ---- pooling: interest = sum_t es * k (slot layout) ----
                tmp = work_pool.tile([128, E, RANKS], f16, tag="tmp")
                nc.vector.tensor_mul(
                    tmp[:, :, 0:RA], stg2[:, :, 0:RA],
                    esp[:, None, 0:RA].broadcast_to([128, E, RA]),
                )
                nc.gpsimd.tensor_mul(
                    tmp[:, :, RA:RANKS], stg2[:, :, RA:RANKS],
                    esp[:, None, RA:RANKS].broadcast_to([128, E, RANKS - RA]),
                )
                acc = work_pool.tile([128, E], f16, tag="acc")
                with nc.allow_low_precision("f16 pooling accum, tol 2e-2"):
                    nc.vector.tensor_reduce(acc[:], tmp[:], AX.X, ALU.add)

                # fold quarters + transpose via pairing matmul
                pit = pm_pool.tile([E, G], f32, tag="pm")
                nc.tensor.matmul(pit[:], acc[:], p2m[:])
                nc.scalar.copy(it512[:, ti * G:(ti + 1) * G], pit[:])

            # ---- head MLP, batched over all tiles ----
            pd1 = ph_pool.tile([128, ntiles * G], f32, tag="ph")
            nc.tensor.matmul(pd1[:], dw1[:], it512[:])
            d1h = h_pool.tile([128, ntiles * G], f16, tag="h16")
            nc.scalar.activation(d1h[:], pd1[:], AF.Relu, bias=db1[:])
            pd2 = ph_pool.tile([E, ntiles * G], f32, tag="ph")
            nc.tensor.matmul(pd2[:], dw2[:], d1h[:])
            d2h = h_pool.tile([E, ntiles * G], f16, tag="h16")
            nc.scalar.activation(d2h[:], pd2[:], AF.Relu, bias=db2[:])
            po = pm_pool.tile([1, ntiles * G], f32, tag="pm")
            nc.tensor.matmul(po[:], ow[:], d2h[:])
            outb = work_pool.tile([1, ntiles * G], f32, tag="outb")
            nc.scalar.activation(outb[:], po[:], AF.Sigmoid, bias=ob[:])
            nc.sync.dma_start(out_d.ap(), outb[:])

    nc.compile()
    return nc


def marshal_inputs(query, keys, emb, att_w1, att_b1, att_w2, att_b2,
                   deep_w1, deep_b1, deep_w2, deep_b2, out_w, out_b,
                   ntiles=NTILES):
    query = np.asarray(query).astype(np.int64)
    keys = np.asarray(keys).astype(np.int64)
    emb16 = np.asarray(emb, dtype=np.float32).astype(np.float16)
    a1 = np.asarray(att_w1, dtype=np.float32)
    Wq, Wk, Wd, Wm = a1[0:64], a1[64:128], a1[128:192], a1[192:256]
    Wpp = np.vstack([Wk - Wd, Wm])
    w1dbl = np.hstack([Wpp, Wpp]).astype(np.float16)
    aug = np.vstack([Wq + Wd, np.asarray(att_b1, np.float32)[None, :]])
    wqdb = np.hstack([aug, aug]).astype(np.float16)
    w2 = np.asarray(att_w2, np.float32)[:, 0]
    w2sel = np.zeros((128, G // 2, G), np.float16)
    for j in range(G // 2):
        w2sel[0:64, j, 2 * j] = w2
        w2sel[64:128, j, 2 * j + 1] = w2
    dw1 = np.asarray(deep_w1, np.float32).astype(np.float16)
    db1 = np.asarray(deep_b1, np.float32).reshape(128, 1)
    dw2 = np.asarray(deep_w2, np.float32).astype(np.float16)
    db2 = np.asarray(deep_b2, np.float32).reshape(64, 1)
    ow = np.asarray(out_w, np.float32).astype(np.float16)
    ob = np.asarray(out_b, np.float32).reshape(1, 1)
    p2 = (np.arange(128)[:, None] % G == np.arange(G)[None, :]).astype(np.float16)

    in_maps = []
    for c in range(NCORES):
        rows = slice(c * BC, c * BC + ntiles * G)
        kt = emb16[keys[rows]].reshape(ntiles, G, T, E)       # [nt,G,T,E]
        qe = emb16[query[rows]].reshape(ntiles, G, E)         # [nt,G,E]
        # kqt[ti, e, gi*T+t] = kt[ti, PI[gi], t, e] (top), * q (bottom)
        ktp = kt[:, PI, :, :]                                 # [nt,32,T,E]
        qep = qe[:, PI, :]                                    # [nt,32,E]
        kh = ktp.transpose(0, 3, 1, 2)                        # [nt,E,32,T]
        kqh = kh * qep.transpose(0, 2, 1)[:, :, :, None]      # *q[e,gi]
        kqt = np.ascontiguousarray(np.concatenate(
            [kh.reshape(ntiles, E, TOK), kqh.reshape(ntiles, E, TOK)], axis=1))
        # slot layout: stg[ti, q*32+g, e, r] = kt[ti, g, 4r+q, e]
        ktr = kt.reshape(ntiles, G, RANKS, NQ, E)
        stg = np.ascontiguousarray(
            ktr.transpose(0, 3, 1, 4, 2).reshape(ntiles, 128, E, RANKS)
        )
        # batched bias rhs: qta[:, par, ti*16+j] = [q(2j+par); 1]
        qT = qe.transpose(0, 2, 1)                            # [nt,E,G]
        qaug = np.concatenate([qT, np.ones((ntiles, 1, G), np.float16)], 1)
        qta = np.ascontiguousarray(
            qaug.reshape(ntiles, 65, G // 2, 2).transpose(1, 3, 0, 2)
            .reshape(65, 2, ntiles * G // 2)
        )
        in_maps.append({
            "kqt": kqt, "stg": stg, "qta": qta,
            "w1dbl": w1dbl, "wqdb": wqdb, "w2sel": w2sel,
            "dw1": dw1, "db1": db1, "dw2": dw2, "db2": db2,
            "ow": ow, "ob": ob, "p2": p2,
        })
    return in_maps


def kernel(**inputs) -> np.ndarray:
    from concourse.bass_utils import run_bass_kernel_spmd

    if "full" not in _nc_cache:
        _nc_cache["full"] = build_nc(NTILES)
    nc = _nc_cache["full"]
    in_maps = marshal_inputs(**inputs)
    res = run_bass_kernel_spmd(nc, in_maps, core_ids=list(range(NCORES)))
    outs = [res.results[c]["out"].reshape(-1) for c in range(NCORES)]
    return np.concatenate(outs).reshape(B, 1).astype(np.float32)


if __name__ == "__main__":
    sys.path.insert(0, "/root/problem")
    import reference
    inputs = {k: np.asarray(v) for k, v in reference.setup_inputs().items()}
    expected = np.asarray(reference.reference(**inputs))
    actual = kernel(**inputs)
    err = np.abs(actual - expected).max() / (np.abs(expected).max() + 1e-12)
    print("Relative error:", err)


# revision 9
# speedup vs baseline: 1.8530x; 1.1340x over previous
"""Trainium2 Bass kernel for DeepInterestNetwork (DIN).

8 cores, data-parallel over batch; each core: 512 rows = 16 tiles of G=32.
Embedding rows are gathered host-side into two SBUF layouts; all
MLP/softmax/pooling compute runs on-device.

Key structure:
  - attention layer1 folds to one K=128 matmul per row via
      inter @ W1 = q@(Wq+Wd) + k@(Wk-Wd) + (q*k)@Wm
    (the q term becomes a per-row-pair bias applied during the relu drain;
    the q*k half of the stack is premultiplied host-side).
  - kqt columns are g-major with even rows first, so each w1 matmul streams
    N=400 (two same-parity rows) with one stationary; a [128, 400] PSUM bank
    then holds a natural row pair per 200-column half for the relu drain.
  - scores: 16 accumulating M=32 matmuls with per-pair column-selector
    stationaries (w2sel) build the full [32, T] score block in one PSUM
    bank -> a single Exp per tile whose accum_out yields the softmax
    denominators for free (scores are tiny, no max-subtraction needed).
  - softmax scatter to the pooling slot layout via 4 stream_shuffles.
  - pooling: slot layout [q*32+g, e, r] keeps the broadcast multiply and the
    X-axis reduction contiguous; the multiply is split DVE/GPSIMD; a pairing
    matmul folds partition quarters and transposes for the head MLP.
  - per-row-pair biases (ctd) are computed for all 16 tiles in one batched
    pre-pass; the head MLP is batched across all 16 tiles at the end.
"""

import numpy as np
import sys

for p in ("/opt/trn_rl_repo", "/opt/trn_rl_repo/concourse"):
    if p not in sys.path:
        sys.path.insert(0, p)

VOCAB, E = 100000, 64
B, T = 4096, 200
NCORES = 8
BC = B // NCORES          # 512 rows per core
G = 32                    # batch rows per tile
NTILES = BC // G          # 16
NQ = 4                    # slot quarters: p = q*32 + g
RANKS = T // NQ           # 50
TOK = G * T               # 6400 columns per tile
RELU_DVE = (1, 4, 7, 10, 13)  # row pairs whose relu runs on DVE

# even-rows-first marshaling order
PI = [2 * i for i in range(16)] + [2 * i + 1 for i in range(16)]

_nc_cache = {}


def build_nc(ntiles=NTILES):
    import concourse.bacc as bacc
    import concourse.mybir as mybir
    import concourse.tile as tile

    f32 = mybir.dt.float32
    f16 = mybir.dt.float16
    AF = mybir.ActivationFunctionType
    ALU = mybir.AluOpType
    AX = mybir.AxisListType

    nc = bacc.Bacc("TRN2", target_bir_lowering=False, debug=False)

    kqt_d = nc.dram_tensor("kqt", [ntiles, 128, TOK], f16, kind="ExternalInput")
    stg_d = nc.dram_tensor("stg", [ntiles, 128, E, RANKS], f16, kind="ExternalInput")
    qta_d = nc.dram_tensor("qta", [65, 2, ntiles * G // 2], f16, kind="ExternalInput")
    w1dbl_d = nc.dram_tensor("w1dbl", [128, 128], f16, kind="ExternalInput")
    wqdb_d = nc.dram_tensor("wqdb", [65, 128], f16, kind="ExternalInput")
    w2sel_d = nc.dram_tensor("w2sel", [128, G // 2, G], f16, kind="ExternalInput")
    dw1_d = nc.dram_tensor("dw1", [E, 128], f16, kind="ExternalInput")
    db1_d = nc.dram_tensor("db1", [128, 1], f32, kind="ExternalInput")
    dw2_d = nc.dram_tensor("dw2", [128, E], f16, kind="ExternalInput")
    db2_d = nc.dram_tensor("db2", [E, 1], f32, kind="ExternalInput")
    ow_d = nc.dram_tensor("ow", [E, 1], f16, kind="ExternalInput")
    ob_d = nc.dram_tensor("ob", [1, 1], f32, kind="ExternalInput")
    p2_d = nc.dram_tensor("p2", [128, G], f16, kind="ExternalInput")
    out_d = nc.dram_tensor("out", [1, ntiles * G], f32, kind="ExternalOutput")

    idmask = list(range(32))
    NC2 = ntiles * G // 2     # 256 row pairs per core

    with tile.TileContext(nc) as tc:
        with tc.tile_pool(name="consts", bufs=1) as consts, \
             tc.tile_pool(name="kqtp", bufs=4) as kqt_pool, \
             tc.tile_pool(name="stgp", bufs=3) as stg_pool, \
             tc.tile_pool(name="work", bufs=2) as work_pool, \
             tc.tile_pool(name="hp", bufs=2) as h_pool, \
             tc.tile_pool(name="ph", bufs=4, space="PSUM") as ph_pool, \
             tc.tile_pool(name="sc", bufs=2, space="PSUM") as sc_pool, \
             tc.tile_pool(name="pm", bufs=2, space="PSUM") as pm_pool:

            # ---- constants ----
            w1dbl = consts.tile([128, 128], f16)
            nc.sync.dma_start(w1dbl[:], w1dbl_d.ap())
            wqdb = consts.tile([65, 128], f16)
            nc.sync.dma_start(wqdb[:], wqdb_d.ap())
            w2sel = consts.tile([128, G // 2, G], f16)
            nc.sync.dma_start(w2sel[:], w2sel_d.ap())
            dw1 = consts.tile([E, 128], f16)
            nc.sync.dma_start(dw1[:], dw1_d.ap())
            db1 = consts.tile([128, 1], f32)
            nc.sync.dma_start(db1[:], db1_d.ap())
            dw2 = consts.tile([128, E], f16)
            nc.sync.dma_start(dw2[:], dw2_d.ap())
            db2 = consts.tile([E, 1], f32)
            nc.sync.dma_start(db2[:], db2_d.ap())
            ow = consts.tile([E, 1], f16)
            nc.sync.dma_start(ow[:], ow_d.ap())
            ob = consts.tile([1, 1], f32)
            nc.sync.dma_start(ob[:], ob_d.ap())
            p2m = consts.tile([128, G], f16)
            nc.sync.dma_start(p2m[:], p2_d.ap())
            qta = consts.tile([65, 2, NC2], f16)
            nc.sync.dma_start(qta[:], qta_d.ap())
            it512 = consts.tile([E, ntiles * G], f16, tag="it512")

            # ---- batched bias pre-pass: ctd_all[:, ti*16+j] = c(pair j) ----
            pca = pm_pool.tile([128, NC2], f32, tag="pm")
            nc.tensor.matmul(
                pca[0:64, :], wqdb[:, 0:64], qta[:, 0, :], tile_position=(0, 0)
            )
            nc.tensor.matmul(
                pca[64:128, :], wqdb[:, 64:128], qta[:, 1, :],
                tile_position=(0, 64),
            )
            ctd_all = consts.tile([128, NC2], f32, tag="ctd")
            nc.scalar.copy(ctd_all[:], pca[:])

            for ti in range(ntiles):
                kqt = kqt_pool.tile([128, TOK], f16, tag="kqt")
                nc.sync.dma_start(kqt[:], kqt_d.ap()[ti])
                stg2 = stg_pool.tile([128, E, RANKS], f16, tag="stg")
                nc.sync.dma_start(stg2[:], stg_d.ap()[ti])

                # ---- attention MLP + scores ----
                # kqt columns: rows in PI order (16 even rows then 16 odd),
                # g-major. w1 matmul k streams two same-parity rows (N=400);
                # psum bank half c holds the natural pair (4k+2c, 4k+2c+1).
                h16 = h_pool.tile([128, G // 2, T], f16, tag="h16")
                scores = sc_pool.tile([G, T], f32, tag="sc")
                for k in range(8):
                    ph = ph_pool.tile([128, 2, T], f32, tag="ph")
                    nc.tensor.matmul(
                        ph[0:64, :, :], w1dbl[:, 0:64],
                        kqt[:, 2 * T * k:2 * T * (k + 1)],
                        tile_position=(0, 0),
                    )
                    nc.tensor.matmul(
                        ph[64:128, :, :], w1dbl[:, 64:128],
                        kqt[:, TOK // 2 + 2 * T * k:TOK // 2 + 2 * T * (k + 1)],
                        tile_position=(0, 64),
                    )
                    for c in range(2):
                        j = 2 * k + c
                        bias = ctd_all[:, ti * (G // 2) + j:ti * (G // 2) + j + 1]
                        if j in RELU_DVE:
                            nc.vector.tensor_scalar(
                                h16[:, j, :], ph[:, c, :], bias, 0.0,
                                ALU.add, ALU.max,
                            )
                        else:
                            nc.scalar.activation(
                                h16[:, j, :], ph[:, c, :], AF.Relu, bias=bias,
                            )
                        nc.tensor.matmul(
                            scores[:], w2sel[:, j, :], h16[:, j, :],
                            start=(j == 0), stop=(j == G // 2 - 1),
                            skip_group_check=True,
                        )

                # ---- softmax (exp + accum denominators + normalize) ----
                es32 = work_pool.tile([G, T], f16, tag="es32")
                dsum = work_pool.tile([G, 1], f32, tag="dsum")
                nc.scalar.activation(es32[:], scores[:], AF.Exp, accum_out=dsum[:])
                rsum = work_pool.tile([G, 1], f32, tag="rsum")
                nc.vector.reciprocal(rsum[:], dsum[:])
                nc.vector.tensor_scalar_mul(es32[:], es32[:], rsum[:])

                # scatter to slot layout esp[q*32+g, r] = es32[g, 4r+q]
                esp = work_pool.tile([128, RANKS], f16, tag="esp")
                esv = es32[:].rearrange("p (r q) -> p q r", q=NQ)
                for q in range(NQ):
                    nc.vector.stream_shuffle(
                        esp[q * G:(q + 1) * G, :], esv[:, q, :], idmask
                    )

                # ---- pooling: interest = sum_t es * k (slot layout) ----
                tmp = work_pool.tile([128, E, RANKS], f16, tag="tmp")
                nc.vector.tensor_mul(
                    tmp[:], stg2[:],
                    esp[:, None, :].broadcast_to([128, E, RANKS]),
                )
                acc = work_pool.tile([128, E], f16, tag="acc")
                with nc.allow_low_precision("f16 pooling accum, tol 2e-2"):
                    nc.vector.tensor_reduce(acc[:], tmp[:], AX.X, ALU.add)

                # fold quarters + transpose via pairing matmul
                pit = pm_pool.tile([E, G], f32, tag="pm")
                nc.tensor.matmul(pit[:], acc[:], p2m[:])
                nc.scalar.copy(it512[:, ti * G:(ti + 1) * G], pit[:])

            # ---- head MLP, batched over all tiles ----
            pd1 = ph_pool.tile([128, ntiles * G], f32, tag="ph")
            nc.tensor.matmul(pd1[:], dw1[:], it512[:])
            d1h = h_pool.tile([128, ntiles * G], f16, tag="h16")
            nc.scalar.activation(d1h[:], pd1[:], AF.Relu, bias=db1[:])
            pd2 = ph_pool.tile([E, ntiles * G], f32, tag="ph")
            nc.tensor.matmul(pd2[:], dw2[:], d1h[:])
            d2h = h_pool.tile([E, ntiles * G], f16, tag="h16")
            nc.scalar.activation(d2h[:], pd2[:], AF.Relu, bias=db2[:])
            po = pm_pool.tile([1, ntiles * G], f32, tag="pm")
            nc.tensor.matmul(po[:], ow[:], d2h[:])
            outb = work_pool.tile([1, ntiles * G], f32, tag="outb")
            nc.scalar.activation(outb[:], po[:], AF.Sigmoid, bias=ob[:])
            nc.sync.dma_start(out_d.ap(), outb[:])

    nc.compile()
    return nc


def marshal_inputs(query, keys, emb, att_w1, att_b1, att_w2, att_b2,
                   deep_w1, deep_b1, deep_w2, deep_b2, out_w, out_b,
                   ntiles=NTILES):
    query = np.asarray(query).astype(np.int64)
    keys = np.asarray(keys).astype(np.int64)
    emb16 = np.asarray(emb, dtype=np.float32).astype(np.float16)
    a1 = np.asarray(att_w1, dtype=np.float32)
    Wq, Wk, Wd, Wm = a1[0:64], a1[64:128], a1[128:192], a1[192:256]
    Wpp = np.vstack([Wk - Wd, Wm])
    w1dbl = np.hstack([Wpp, Wpp]).astype(np.float16)
    aug = np.vstack([Wq + Wd, np.asarray(att_b1, np.float32)[None, :]])
    wqdb = np.hstack([aug, aug]).astype(np.float16)
    w2 = np.asarray(att_w2, np.float32)[:, 0]
    w2sel = np.zeros((128, G // 2, G), np.float16)
    for j in range(G // 2):
        w2sel[0:64, j, 2 * j] = w2
        w2sel[64:128, j, 2 * j + 1] = w2
    dw1 = np.asarray(deep_w1, np.float32).astype(np.float16)
    db1 = np.asarray(deep_b1, np.float32).reshape(128, 1)
    dw2 = np.asarray(deep_w2, np.float32).astype(np.float16)
    db2 = np.asarray(deep_b2, np.float32).reshape(64, 1)
    ow = np.asarray(out_w, np.float32).astype(np.float16)
    ob = np.asarray(out_b, np.float32).reshape(1, 1)
    p2 = (np.arange(128)[:, None] % G == np.arange(G)[None, :]).astype(np.float16)

    in_maps = []
    for c in range(NCORES):
        rows = slice(c * BC, c * BC + ntiles * G)
        kt = emb16[keys[rows]].reshape(ntiles, G, T, E)       # [nt,G,T,E]
        qe = emb16[query[rows]].reshape(ntiles, G, E)         # [nt,G,E]
        # kqt[ti, e, gi*T+t] = kt[ti, PI[gi], t, e] (top), * q (bottom)
        ktp = kt[:, PI, :, :]                                 # [nt,32,T,E]
        qep = qe[:, PI, :]                                    # [nt,32,E]
        kh = ktp.transpose(0, 3, 1, 2)                        # [nt,E,32,T]
        kqh = kh * qep.transpose(0, 2, 1)[:, :, :, None]      # *q[e,gi]
        kqt = np.ascontiguousarray(np.concatenate(
            [kh.reshape(ntiles, E, TOK), kqh.reshape(ntiles, E, TOK)], axis=1))
        # slot layout: stg[ti, q*32+g, e, r] = kt[ti, g, 4r+q, e]
        ktr = kt.reshape(ntiles, G, RANKS, NQ, E)
        stg = np.ascontiguousarray(
            ktr.transpose(0, 3, 1, 4, 2).reshape(ntiles, 128, E, RANKS)
        )
        # batched bias rhs: qta[:, par, ti*16+j] = [q(2j+par); 1]
        qT = qe.transpose(0, 2, 1)                            # [nt,E,G]
        qaug = np.concatenate([qT, np.ones((ntiles, 1, G), np.float16)], 1)
        qta = np.ascontiguousarray(
            qaug.reshape(ntiles, 65, G // 2, 2).transpose(1, 3, 0, 2)
            .reshape(65, 2, ntiles * G // 2)
        )
        in_maps.append({
            "kqt": kqt, "stg": stg, "qta": qta,
            "w1dbl": w1dbl, "wqdb": wqdb, "w2sel": w2sel,
            "dw1": dw1, "db1": db1, "dw2": dw2, "db2": db2,
            "ow": ow, "ob": ob, "p2": p2,
        })
    return in_maps


def kernel(**inputs) -> np.ndarray:
    from concourse.bass_utils import run_bass_kernel_spmd

    if "full" not in _nc_cache:
        _nc_cache["full"] = build_nc(NTILES)
    nc = _nc_cache["full"]
    in_maps = marshal_inputs(**inputs)
    res = run_bass_kernel_spmd(nc, in_maps, core_ids=list(range(NCORES)))
    outs = [res.results[c]["out"].reshape(-1) for c in range(NCORES)]
    return np.concatenate(outs).reshape(B, 1).astype(np.float32)


if __name__ == "__main__":
    sys.path.insert(0, "/root/problem")
    import reference
    inputs = {k: np.asarray(v) for k, v in reference.setup_inputs().items()}
    expected = np.asarray(reference.reference(**inputs))
    actual = kernel(**inputs)
    err = np.abs(actual - expected).max() / (np.abs(expected).max() + 1e-12)
    print("Relative error:", err)
